# revision 26
# baseline (speedup 1.0000x reference)
"""CrossAttention Trainium2 kernel (8 NeuronCores, SPMD).

Sharding: 8 cores = batch(2) x query-block(4 x 1024). Each core computes a
[1024, 1024] slice of the output; no cross-core communication.

Reference math (per core, M=1024 query tokens, Skv=1024, D=1024, H=16, hd=64):
  q = hs @ Wq ; k = enc @ Wk ; v = enc @ Wv
  per-head LN(q), LN(k) over hd; scores = LN(q) @ LN(k)^T / sqrt(hd)
  out = softmax(scores) @ v ; return out @ Wo

Key structure (vs. the direct formulation):
  - LN mean-centering is folded into Wq/Wk on the host (exact).
  - With unit gains / zero biases, the K-side 1/std scale commutes into the
    exp: exp(s * rinv_k / 8) via the activation's per-partition scale, so
    K LayerNorm costs no broadcast/multiply passes at all.
  - The Q-side 1/std is broadcast with GpSimd partition_broadcast (not PE
    selector matmuls) and applied by one vector multiply per feature block.
  - V carries an extra ones-column so softmax denominators fall out of the
    attention-value matmul; they are normalized per head-pair in a trailing
    pipeline (GpSimd row copy -> approx reciprocal -> GpSimd broadcast ->
    vector multiply) that never blocks the PE.
  - Emission order keeps the PE streaming continuously (projection phases
    overlap the LN dependency chains; V-projection is split across P1 and
    the first attention iteration) to avoid HAM clock-throttle windows.
"""

import numpy as np
import ml_dtypes
from contextlib import ExitStack

import concourse.bass as bass
import concourse.tile as tile
from concourse import bacc, mybir
from concourse.bass_utils import run_bass_kernel_spmd
from concourse.dve_ops import (
    RECIP_APPROX_FAST_CONSTS,
    RECIPROCAL_APPROX_FAST,
    RECIPROCAL_APPROX_NR,
)

BF = mybir.dt.bfloat16
F32 = mybir.dt.float32
F32R = mybir.dt.float32r

D = 1024      # model dim
H = 16        # heads
HD = 64       # head dim
M = 1024      # query tokens per core
SKV = 1024    # kv tokens (one batch)
B = 2
SQ = 4096
NCORES = 8
LN_EPS = 1e-5

_cache = {}


def _recip_approx_r(nc, out, in_, scratch):
    """~2 ULP approximate 1/in_ written as FP32R (rounds on write, so the
    result is legal as an f32r matmul operand). in_/scratch are fp32."""
    c = RECIP_APPROX_FAST_CONSTS
    nc.vector._custom_dve(RECIPROCAL_APPROX_FAST, out=scratch, in0=in_,
                          s0=c["s0"], s1=c["s1"], imm2=c["imm2"])
    nc.vector._custom_dve(RECIPROCAL_APPROX_NR, out=out, in0=in_,
                          in1=scratch, s0=2.0)


def _selector_constants():
    # sel16[d][p, j]: 1 if head j == 2d + p//64  (sum-over-head-partitions lhsT)
    sel16 = np.zeros((8, 128, H), np.float32)
    for d in range(8):
        for p in range(128):
            sel16[d, p, 2 * d + p // 64] = 1.0
    # selB[d][j, p]: 1 if head j == 2d + p//64  (broadcast-to-head-partitions)
    selB = np.transpose(sel16, (0, 2, 1)).copy()
    # selE[0][h][j]: identity — scatter a [1, CH] row into head h of [16, CH]
    selE = np.eye(H, dtype=np.float32)[None, :, :]
    return sel16, selB, selE


def _emit(ctx: ExitStack, tc, t, fastq, fastk, has_bq, has_bk):
    nc = tc.nc

    persist = ctx.enter_context(tc.tile_pool(name="persist", bufs=1))

    # ---- persistent SBUF tensors (stacked [128, 8, 1024] layout) ----
    hst = persist.tile([128, 8, M], BF, tag="hst")       # hs^T  (feature-major)
    enct = persist.tile([128, 8, SKV], BF, tag="enct")   # enc^T (feature-major)
    wq = persist.tile([128, 8, D], BF, tag="wq")
    wk = persist.tile([128, 8, D], BF, tag="wk")
    wv = persist.tile([128, 8, D], BF, tag="wv")
    wo = persist.tile([128, 8, D], BF, tag="wo")
    qtln = persist.tile([128, 8, M], BF, tag="qtln")     # LN(q)^T feature-major
    ktln = persist.tile([128, 8, SKV], BF, tag="ktln")   # k_c^T feature-major
    vaug = persist.tile([128, 8, H, HD + 1], BF, tag="vaug")  # [kv, h, V|1]
    aout = hst  # hs^T is dead after Q-projection; reuse for attn out^T
    sel16_sb = persist.tile([128, 8, H], F32R, tag="sel16_sb")
    selB_sb = persist.tile([16, 8, 128], F32R, tag="selB_sb")
    selE_sb = persist.tile([1, H, H], F32R, tag="selE_sb")
    eye16_sb = persist.tile([16, 16], F32, tag="eye16_sb")
    invs = persist.tile([16, 2, 512], F32R, tag="invs")   # 1/softmax-sums
    scr16 = persist.tile([16, 2, 512], F32, tag="scr16")  # recip scratch
    rinvq = persist.tile([16, 2, 512], F32R, tag="rinvq")  # 1/std (hq, tok)
    rinvk8 = persist.tile([128, 8, 16], F32, tag="rinvk8")  # rinv_k/8, kv-major
    scr128 = persist.tile([128, 8, 16], F32, tag="scr128")
    eps_sb = persist.tile([16, 1], F32, tag="eps_sb")
    eps64_sb = persist.tile([16, 1], F32, tag="eps64_sb")
    nc.vector.memset(eps_sb[:, :], LN_EPS)
    nc.vector.memset(eps64_sb[:, :], HD * LN_EPS)
    nc.vector.memset(vaug[:, :, :, HD:HD + 1], 1.0)

    generic = (not fastq) or (not fastk) or has_bq or has_bk
    gq_sb = (persist.tile([128, 8], F32, tag="gq_sb", name="gq_sb")
             if generic else None)
    gk_sb = (persist.tile([128, 8], F32, tag="gk_sb", name="gk_sb")
             if generic else None)
    bq_sb = (persist.tile([128, 8], F32, tag="bq_sb", name="bq_sb")
             if has_bq else None)
    bk_sb = (persist.tile([128, 8], F32, tag="bk_sb", name="bk_sb")
             if has_bk else None)
    rinvk_t = (persist.tile([16, 2, 512], F32R, tag="rinvk_t", name="rinvk_t")
               if not fastk else None)

    # ---- loads (k-granular so compute starts as slices land) ----
    for k in range(8):
        nc.sync.dma_start(enct[:, k, :], t["encT"][k * 128:(k + 1) * 128, :])
        nc.sync.dma_start(wk[:, k, :], t["wk"][k * 128:(k + 1) * 128, :])
        if k == 0:
            nc.sync.dma_start(sel16_sb[:, :, :],
                              t["sel16"].rearrange("d p j -> p d j"))
            nc.sync.dma_start(selB_sb[:, :, :],
                              t["selB"].rearrange("d j p -> j d p"))
            nc.sync.dma_start(selE_sb[:, :, :], t["selE"])
            nc.sync.dma_start(eye16_sb[:, :], t["eye16"])
    for k in range(8):
        nc.sync.dma_start(hst[:, k, :], t["hsT"][k * 128:(k + 1) * 128, :])
        nc.sync.dma_start(wq[:, k, :], t["wq"][k * 128:(k + 1) * 128, :])
    for k in range(8):
        nc.sync.dma_start(wv[:, k, :], t["wv"][k * 128:(k + 1) * 128, :])
    for k in range(8):
        nc.sync.dma_start(wo[:, k, :], t["wo"][k * 128:(k + 1) * 128, :])
    if generic:
        nc.sync.dma_start(gq_sb[:, :], t["gq"].rearrange("(d p) -> p d", p=128))
        nc.sync.dma_start(gk_sb[:, :], t["gk"].rearrange("(d p) -> p d", p=128))
    if has_bq:
        nc.sync.dma_start(bq_sb[:, :], t["bq"].rearrange("(d p) -> p d", p=128))
    if has_bk:
        nc.sync.dma_start(bk_sb[:, :], t["bk"].rearrange("(d p) -> p d", p=128))

    sq_pool = ctx.enter_context(tc.tile_pool(name="sq_pool", bufs=2))
    srow_pool = ctx.enter_context(tc.tile_pool(name="srow_pool", bufs=2))

    p1 = ExitStack()
    ps_proj = p1.enter_context(tc.tile_pool(name="ps_proj", bufs=4,
                                            space="PSUM"))
    ps_ssq = p1.enter_context(tc.tile_pool(name="ps_ssq", bufs=2,
                                           space="PSUM"))
    # tag-shared ring: ssq_k -> ssq_q -> tps rotate through 2 x 2-bank slots
    tmp16 = p1.enter_context(tc.tile_pool(name="tmp16", bufs=2))
    stdq = tmp16.tile([16, 2, 512], F32, tag="std16", name="stdq")
    std8k = tmp16.tile([16, 2, 512], F32, tag="std16", name="std8k")

    def proj(w_sb, x_sb, dst, ssq_ps):
        # dst[:, d, :] = (x^T W)[d-block]^T staged bf16; ssq_ps[16, 2, 512]
        # accumulates per-head sums of squares via the sel16 selector.
        for d in range(8):
            accs = [ps_proj.tile([128, 512], F32, tag="acc", name=f"acc{c}")
                    for c in range(2)]
            for k in range(8):
                for c in range(2):
                    nc.tensor.matmul(
                        accs[c][:, :],
                        lhsT=w_sb[:, k, d * 128:(d + 1) * 128],
                        rhs=x_sb[:, k, c * 512:(c + 1) * 512],
                        start=(k == 0), stop=(k == 7),
                    )
            for c in range(2):
                staged = dst[:, d, c * 512:(c + 1) * 512]
                nc.scalar.copy(staged, accs[c][:, :])
                sq = sq_pool.tile([128, 512], F32R)
                nc.vector.tensor_mul(sq[:, :], staged, accs[c][:, :])
                nc.tensor.matmul(
                    ssq_ps[:, c, :],
                    lhsT=sel16_sb[:, d, :],
                    rhs=sq[:, :],
                    start=(d == 0), stop=(d == 7),
                    skip_group_check=True,
                )

    def apply_rb(dst, rb_ps, rinv_sb, g_sb, b_sb, fast):
        # dst[:, d, :] *= selB-broadcast(rinv rows) (*g +b in the generic path)
        for d in range(8):
            rb = rb_ps.tile([128, 2, 512], F32, tag="ssq", name=f"rb{d}")
            for c in range(2):
                nc.tensor.matmul(
                    rb[:, c, :],
                    lhsT=selB_sb[:, d, :],
                    rhs=rinv_sb[:, c, :],
                    start=True, stop=True,
                )
            for c in range(2):
                dslice = dst[:, d, c * 512:(c + 1) * 512]
                if fast:
                    nc.vector.tensor_mul(dslice, dslice, rb[:, c, :])
                else:
                    nc.vector.scalar_tensor_tensor(
                        out=dslice, in0=dslice, scalar=g_sb[:, d:d + 1],
                        in1=rb[:, c, :],
                        op0=mybir.AluOpType.mult, op1=mybir.AluOpType.mult,
                    )
                if b_sb is not None:
                    nc.vector.tensor_scalar_add(dslice, dslice,
                                                b_sb[:, d:d + 1])

    # ---- P1: K projection ----
    ssq_k = ps_ssq.tile([16, 2, 512], F32, tag="ssq", name="ssq_k")
    proj(wk, enct, ktln, ssq_k)
    if fastk:
        # std8k = sqrt(ssq + 64*eps) = 8*std; exp scale uses 1/std8k = rinv/8
        nc.scalar.activation(std8k[:, :, :], ssq_k[:, :, :],
                             mybir.ActivationFunctionType.Sqrt,
                             bias=eps64_sb[:, :], scale=1.0)
    else:
        nc.scalar.activation(std8k[:, :, :], ssq_k[:, :, :],
                             mybir.ActivationFunctionType.Sqrt,
                             bias=eps_sb[:, :], scale=1.0 / HD)
        _recip_approx_r(nc, rinvk_t[:, :, :], std8k[:, :, :],
                        scr16[:, :, :])

    # ---- P1: Q projection (K's post-chain overlaps these matmuls) ----
    ssq_q = ps_ssq.tile([16, 2, 512], F32, tag="ssq", name="ssq_q")
    proj(wq, hst, qtln, ssq_q)
    nc.scalar.activation(stdq[:, :, :], ssq_q[:, :, :],
                         mybir.ActivationFunctionType.Sqrt,
                         bias=eps_sb[:, :], scale=1.0 / HD)
    _recip_approx_r(nc, rinvq[:, :, :], stdq[:, :, :], scr16[:, :, :])

    if fastk:
        # transpose std8k -> [kv, (vblock, head)] then one cheap reciprocal
        tps = ps_ssq.tile([128, 8, 16], F32, tag="ssq", name="tps")
        for vb in range(8):
            c, o = divmod(vb, 4)
            nc.tensor.transpose(tps[:, vb, :],
                                std8k[:, c, o * 128:(o + 1) * 128],
                                eye16_sb[:, :])
        nc.vector.reciprocal_approx_accurate(
            rinvk8[:, :, :], tps[:, :, :], scr128[:, :, :])
    else:
        apply_rb(ktln, ps_ssq, rinvk_t, gk_sb, bk_sb, fast=False)

    def vproj(tt_lo, tt_hi, pool, tag):
        for tt in range(tt_lo, tt_hi):
            for c in range(2):
                vacc = pool.tile([128, 512], F32, tag=tag, name="vacc")
                for k in range(8):
                    nc.tensor.matmul(
                        vacc[:, :],
                        lhsT=enct[:, k, tt * 128:(tt + 1) * 128],
                        rhs=wv[:, k, c * 512:(c + 1) * 512],
                        start=(k == 0), stop=(k == 7),
                    )
                dst = vaug[:, tt, 8 * c:8 * (c + 1), 0:HD]
                nc.vector.tensor_copy(
                    dst, vacc[:, :].rearrange("p (h e) -> p h e", e=HD))

    # V-projection head start: covers the Q LN chain so the PE never idles.
    vproj(0, 3, ps_proj, "acc")

    # Q-side LN apply; scores for head-pair p only need block d==p, so the
    # d-loop pipelines ahead of attention.
    apply_rb(qtln, ps_ssq, rinvq, gq_sb, bq_sb, fast=fastq)
    p1.close()

    # ---- P2 pools (PSUM freed by p1.close(): sc 2 + av 4 + sums 2 banks) --
    at_pool = ctx.enter_context(tc.tile_pool(name="at_pool", bufs=3))
    p2 = ExitStack()
    ps_sc = p2.enter_context(tc.tile_pool(name="ps_sc", bufs=1, space="PSUM"))
    ps_av = p2.enter_context(tc.tile_pool(name="ps_av", bufs=2, space="PSUM"))
    ps_sum = p2.enter_context(tc.tile_pool(name="ps_sum", bufs=1,
                                           space="PSUM"))
    sums_ps = ps_sum.tile([16, 2, 512], F32, tag="sums")

    # ---- P2: attention, software-pipelined (AV lags exp by one tile) ----
    pend_av = None

    def flush_av():
        nonlocal pend_av
        if pend_av is None:
            return
        avs, j, v, at = pend_av
        h = 2 * avs["p"] + j
        for qh in range(2):
            nc.tensor.matmul(
                avs[j][:, qh, :],
                lhsT=vaug[:, v, h, :],
                rhs=at[:, qh, :],
                start=(v == 0), stop=(v == 7),
                skip_group_check=True,
            )
        pend_av = None

    for p in range(8):
        avs = {j: ps_av.tile([HD + 1, 2, 512], F32, tag="av", name=f"av{j}")
               for j in range(2)}
        avs["p"] = p
        for v in range(8):
            if p == 0 and v >= 3:
                vproj(v, v + 1, ps_sc, "sc")  # finish V just-in-time
            for j in range(2):
                h = 2 * p + j
                scs = ps_sc.tile([128, 2, 512], F32, tag="sc")
                for qh in range(2):
                    nc.tensor.matmul(
                        scs[:, qh, :],
                        lhsT=ktln[j * 64:(j + 1) * 64, p,
                                  v * 128:(v + 1) * 128],
                        rhs=qtln[j * 64:(j + 1) * 64, p,
                                 qh * 512:(qh + 1) * 512],
                        start=True, stop=True,
                    )
                flush_av()
                at = at_pool.tile([128, 2, 512], BF)
                scale = rinvk8[:, v, h:h + 1] if fastk else 0.125
                nc.scalar.activation(
                    at[:, :, :], scs[:, :, :],
                    mybir.ActivationFunctionType.Exp, scale=scale,
                )
                pend_av = (avs, j, v, at)
        flush_av()
        for j in range(2):
            h = 2 * p + j
            nc.vector.tensor_copy(aout[j * 64:(j + 1) * 64, p, :],
                                  avs[j][0:HD, :, :])
            srow = srow_pool.tile([1, 2, 512], F32R)
            with nc.allow_low_precision(reason="f32r softmax sum staging"):
                nc.vector.tensor_copy(srow[:, :, :], avs[j][HD:HD + 1, :, :])
            for c in range(2):
                nc.tensor.matmul(
                    sums_ps[:, c, :],
                    lhsT=selE_sb[:, h, :],
                    rhs=srow[:, c, :],
                    start=(h == 0), stop=(h == 15),
                    skip_group_check=True,
                )

    # ---- P2.5/P3: normalize by softmax sums and project, per token-half ---
    for c in range(2):
        _recip_approx_r(nc, invs[:, c, :], sums_ps[:, c, :], scr16[:, c, :])
        for d in range(8):
            rbi = ps_sc.tile([128, 512], F32, tag="sc", name="rbi")
            nc.tensor.matmul(
                rbi[:, :],
                lhsT=selB_sb[:, d, :],
                rhs=invs[:, c, :],
                start=True, stop=True,
            )
            sl = aout[:, d, c * 512:(c + 1) * 512]
            nc.vector.tensor_mul(sl, sl, rbi[:, :])
    p2.close()

    out_pool = ctx.enter_context(tc.tile_pool(name="out_pool", bufs=4))
    with tc.tile_pool(name="ps_out", bufs=4, space="PSUM") as ps_out:
        for c in range(2):
            for tt in range(4 * c, 4 * (c + 1)):
                accs = {cc: ps_out.tile([128, 512], F32, tag="oacc",
                                        name=f"oacc{cc}") for cc in range(2)}
                for k in range(8):
                    for cc in range(2):
                        nc.tensor.matmul(
                            accs[cc][:, :],
                            lhsT=aout[:, k, tt * 128:(tt + 1) * 128],
                            rhs=wo[:, k, cc * 512:(cc + 1) * 512],
                            start=(k == 0), stop=(k == 7),
                        )
                for cc in range(2):
                    ot = out_pool.tile([128, 512], F32)
                    nc.scalar.copy(ot[:, :], accs[cc][:, :])
                    nc.sync.dma_start(
                        t["out"][tt * 128:(tt + 1) * 128,
                                 cc * 512:(cc + 1) * 512],
                        ot[:, :],
                    )


def _build(fastq, fastk, has_bq, has_bk):
    key = (fastq, fastk, has_bq, has_bk)
    if key in _cache:
        return _cache[key]
    nc = bacc.Bacc("TRN2", target_bir_lowering=False, debug=False,
                   num_devices=NCORES)
    t = {}

    def inp(name, shape, dt):
        t[name] = nc.dram_tensor(name, list(shape), dt, kind="ExternalInput").ap()

    inp("hsT", (D, M), BF)
    inp("encT", (D, SKV), BF)
    inp("wq", (D, D), BF)
    inp("wk", (D, D), BF)
    inp("wv", (D, D), BF)
    inp("wo", (D, D), BF)
    if (not fastq) or (not fastk) or has_bq or has_bk:
        inp("gq", (D,), F32)
        inp("gk", (D,), F32)
    if has_bq:
        inp("bq", (D,), F32)
    if has_bk:
        inp("bk", (D,), F32)
    inp("sel16", (8, 128, H), F32R)
    inp("selB", (8, H, 128), F32R)
    inp("selE", (1, H, H), F32R)
    inp("eye16", (16, 16), F32)
    t["out"] = nc.dram_tensor("out", [M, D], F32, kind="ExternalOutput").ap()

    with tile.TileContext(nc) as tc:
        with ExitStack() as ctx:
            _emit(ctx, tc, t, fastq, fastk, has_bq, has_bk)
    nc.finalize()
    _cache[key] = nc
    return nc


def _center_fold(W):
    # Fold per-head output-column mean removal into the weight matrix (exact).
    Wr = np.asarray(W, np.float32).reshape(D, H, HD)
    return (Wr - Wr.mean(axis=2, keepdims=True)).reshape(D, D)


def kernel(hidden_states, encoder_hidden_states, Wq, Wk, Wv, Wo,
           gq, bq, gk, bk, _trace=False):
    hs = np.asarray(hidden_states, np.float32)
    enc = np.asarray(encoder_hidden_states, np.float32)
    gq = np.asarray(gq, np.float32)
    gk = np.asarray(gk, np.float32)
    bq = np.asarray(bq, np.float32)
    bk = np.asarray(bk, np.float32)
    has_bq = bool(np.any(bq != 0))
    has_bk = bool(np.any(bk != 0))
    fastq = bool(np.all(gq == 1.0)) and not has_bq
    fastk = bool(np.all(gk == 1.0)) and not has_bk
    nc = _build(fastq, fastk, has_bq, has_bk)

    bf = ml_dtypes.bfloat16
    sel16, selB, selE = _selector_constants()
    common = {
        "wq": _center_fold(Wq).astype(bf),
        "wk": _center_fold(Wk).astype(bf),
        "wv": np.asarray(Wv, np.float32).astype(bf),
        "wo": np.asarray(Wo, np.float32).astype(bf),
        "sel16": sel16, "selB": selB, "selE": selE,
        "eye16": np.eye(16, dtype=np.float32),
    }
    if (not fastq) or (not fastk) or has_bq or has_bk:
        common["gq"] = np.tile(gq, H)
        common["gk"] = np.tile(gk, H)
    if has_bq:
        common["bq"] = np.tile(bq, H)
    if has_bk:
        common["bk"] = np.tile(bk, H)

    in_maps = []
    for core in range(NCORES):
        b, qb = divmod(core, 4)
        hsT = np.ascontiguousarray(
            hs[b, qb * M:(qb + 1) * M, :].T).astype(bf)
        encT = np.ascontiguousarray(enc[b].T).astype(bf)
        in_maps.append({**common, "hsT": hsT, "encT": encT})

    res = run_bass_kernel_spmd(nc, in_maps, list(range(NCORES)), trace=_trace)

    out = np.empty((B, SQ, D), np.float32)
    for core in range(NCORES):
        b, qb = divmod(core, 4)
        out[b, qb * M:(qb + 1) * M, :] = res.results[core]["out"]
    kernel.last_exec_time_ns = res.exec_time_ns
    kernel.last_results = res
    return out


# revision 29
# speedup vs baseline: 1.3174x; 1.3174x over previous
"""CrossAttention Trainium2 kernel (8 NeuronCores, SPMD).

Sharding: 8 cores = batch(2) x query-block(4 x 1024). Each core computes a
[1024, 1024] slice of the output; no cross-core communication.

Reference math (per core, M=1024 query tokens, Skv=1024, D=1024, H=16, hd=64):
  q = hs @ Wq ; k = enc @ Wk ; v = enc @ Wv
  per-head LN(q), LN(k) over hd; scores = LN(q) @ LN(k)^T / sqrt(hd)
  out = softmax(scores) @ v ; return out @ Wo

Key structure (vs. the direct formulation):
  - LN mean-centering is folded into Wq/Wk on the host (exact).
  - With unit gains / zero biases, the K-side 1/std scale commutes into the
    exp: exp(s * rinv_k / 8) via the activation's per-partition scale, so
    K LayerNorm costs no broadcast/multiply passes at all.
  - The Q-side 1/std is broadcast to feature partitions with selector
    matmuls and applied by vector multiplies, interleaved with the start of
    the V-projection so the PE never idles (idle windows trip the HAM clock
    throttle to 1.2 GHz).
  - V carries an extra ones-column so softmax denominators fall out of the
    attention-value matmul; per head-pair they are scattered into a [16, M]
    table, reciprocated with a fast approx reciprocal, broadcast back and
    applied per token-half so the output projection overlaps the tail.
  - Emission order keeps the PE streaming continuously (projection phases
    overlap the LN dependency chains; V-projection is split across P1 and
    the first attention iteration) to avoid HAM clock-throttle windows.
"""

import numpy as np
import ml_dtypes
from contextlib import ExitStack

import concourse.bass as bass
import concourse.tile as tile
from concourse import bacc, mybir
from concourse.bass_utils import run_bass_kernel_spmd
from concourse.dve_ops import (
    RECIP_APPROX_FAST_CONSTS,
    RECIPROCAL_APPROX_FAST,
    RECIPROCAL_APPROX_NR,
)

BF = mybir.dt.bfloat16
F32 = mybir.dt.float32
F32R = mybir.dt.float32r

D = 1024      # model dim
H = 16        # heads
HD = 64       # head dim
M = 1024      # query tokens per core
SKV = 1024    # kv tokens (one batch)
B = 2
SQ = 4096
NCORES = 8
LN_EPS = 1e-5

_cache = {}


def _recip_approx_r(nc, out, in_, scratch):
    """~2 ULP approximate 1/in_ written as FP32R (rounds on write, so the
    result is legal as an f32r matmul operand). in_/scratch are fp32."""
    c = RECIP_APPROX_FAST_CONSTS
    nc.vector._custom_dve(RECIPROCAL_APPROX_FAST, out=scratch, in0=in_,
                          s0=c["s0"], s1=c["s1"], imm2=c["imm2"])
    nc.vector._custom_dve(RECIPROCAL_APPROX_NR, out=out, in0=in_,
                          in1=scratch, s0=2.0)


def _selector_constants():
    # sel16[d][p, j]: 1 if head j == 2d + p//64  (sum-over-head-partitions lhsT)
    sel16 = np.zeros((8, 128, H), np.float32)
    for d in range(8):
        for p in range(128):
            sel16[d, p, 2 * d + p // 64] = 1.0
    # selB[d][j, p]: 1 if head j == 2d + p//64  (broadcast-to-head-partitions)
    selB = np.transpose(sel16, (0, 2, 1)).copy()
    # selE[0][h][j]: identity — scatter a [1, CH] row into head h of [16, CH]
    selE = np.eye(H, dtype=np.float32)[None, :, :]
    return sel16, selB, selE


def _emit(ctx: ExitStack, tc, t, fastq, fastk, has_bq, has_bk):
    nc = tc.nc

    persist = ctx.enter_context(tc.tile_pool(name="persist", bufs=1))

    # ---- persistent SBUF tensors (stacked [128, 8, 1024] layout) ----
    hst = persist.tile([128, 8, M], BF, tag="hst")       # hs^T  (feature-major)
    enct = persist.tile([128, 8, SKV], BF, tag="enct")   # enc^T (feature-major)
    wq = persist.tile([128, 8, D], BF, tag="wq")
    wk = persist.tile([128, 8, D], BF, tag="wk")
    wv = persist.tile([128, 8, D], BF, tag="wv")
    wo = persist.tile([128, 8, D], BF, tag="wo")
    qtln = persist.tile([128, 8, M], BF, tag="qtln")     # LN(q)^T feature-major
    ktln = persist.tile([128, 8, SKV], BF, tag="ktln")   # k_c^T feature-major
    vaug = persist.tile([128, 8, H, HD + 1], BF, tag="vaug")  # [kv, h, V|1]
    aout = hst  # hs^T is dead after Q-projection; reuse for attn out^T
    sel16_sb = persist.tile([128, 8, H], F32R, tag="sel16_sb")
    selB_sb = persist.tile([16, 8, 128], F32R, tag="selB_sb")
    selE_sb = persist.tile([1, H, H], F32R, tag="selE_sb")
    eye16_sb = persist.tile([16, 16], F32, tag="eye16_sb")
    sums_sb = persist.tile([16, 2, 512], F32, tag="sums_sb")
    invs = persist.tile([16, 2, 512], F32R, tag="invs")   # 1/softmax-sums
    scr16 = persist.tile([16, 2, 512], F32, tag="scr16")  # recip scratch
    rinvq = persist.tile([16, 2, 512], F32R, tag="rinvq")  # 1/std (hq, tok)
    rinvk8 = persist.tile([128, 8, 16], F32, tag="rinvk8")  # rinv_k/8, kv-major
    scr128 = persist.tile([128, 8, 16], F32, tag="scr128")
    eps_sb = persist.tile([16, 1], F32, tag="eps_sb")
    eps64_sb = persist.tile([16, 1], F32, tag="eps64_sb")
    nc.vector.memset(eps_sb[:, :], LN_EPS)
    nc.vector.memset(eps64_sb[:, :], HD * LN_EPS)
    nc.vector.memset(vaug[:, :, :, HD:HD + 1], 1.0)
    nc.vector.memset(sums_sb[:, :, :], 0.0)

    generic = (not fastq) or (not fastk) or has_bq or has_bk
    gq_sb = (persist.tile([128, 8], F32, tag="gq_sb", name="gq_sb")
             if generic else None)
    gk_sb = (persist.tile([128, 8], F32, tag="gk_sb", name="gk_sb")
             if generic else None)
    bq_sb = (persist.tile([128, 8], F32, tag="bq_sb", name="bq_sb")
             if has_bq else None)
    bk_sb = (persist.tile([128, 8], F32, tag="bk_sb", name="bk_sb")
             if has_bk else None)
    rinvk_t = (persist.tile([16, 2, 512], F32R, tag="rinvk_t", name="rinvk_t")
               if not fastk else None)

    # ---- loads (k-granular so compute starts as slices land) ----
    for k in range(8):
        nc.sync.dma_start(enct[:, k, :], t["encT"][k * 128:(k + 1) * 128, :])
        nc.sync.dma_start(wk[:, k, :], t["wk"][k * 128:(k + 1) * 128, :])
        if k == 0:
            nc.sync.dma_start(sel16_sb[:, :, :],
                              t["sel16"].rearrange("d p j -> p d j"))
            nc.sync.dma_start(selB_sb[:, :, :],
                              t["selB"].rearrange("d j p -> j d p"))
            nc.sync.dma_start(selE_sb[:, :, :], t["selE"])
            nc.sync.dma_start(eye16_sb[:, :], t["eye16"])
    for k in range(8):
        nc.sync.dma_start(hst[:, k, :], t["hsT"][k * 128:(k + 1) * 128, :])
        nc.sync.dma_start(wq[:, k, :], t["wq"][k * 128:(k + 1) * 128, :])
    for k in range(8):
        nc.sync.dma_start(wv[:, k, :], t["wv"][k * 128:(k + 1) * 128, :])
    for k in range(8):
        nc.sync.dma_start(wo[:, k, :], t["wo"][k * 128:(k + 1) * 128, :])
    if generic:
        nc.sync.dma_start(gq_sb[:, :], t["gq"].rearrange("(d p) -> p d", p=128))
        nc.sync.dma_start(gk_sb[:, :], t["gk"].rearrange("(d p) -> p d", p=128))
    if has_bq:
        nc.sync.dma_start(bq_sb[:, :], t["bq"].rearrange("(d p) -> p d", p=128))
    if has_bk:
        nc.sync.dma_start(bk_sb[:, :], t["bk"].rearrange("(d p) -> p d", p=128))

    sq_pool = ctx.enter_context(tc.tile_pool(name="sq_pool", bufs=2))
    srow_pool = ctx.enter_context(tc.tile_pool(name="srow_pool", bufs=2))

    p1 = ExitStack()
    ps_proj = p1.enter_context(tc.tile_pool(name="ps_proj", bufs=4,
                                            space="PSUM"))
    ps_ssq = p1.enter_context(tc.tile_pool(name="ps_ssq", bufs=2,
                                           space="PSUM"))
    # tag-shared ring: ssq_k -> ssq_q -> tps rotate through 2 x 2-bank slots
    tmp16 = p1.enter_context(tc.tile_pool(name="tmp16", bufs=2))
    stdq = tmp16.tile([16, 2, 512], F32, tag="std16", name="stdq")
    std8k = tmp16.tile([16, 2, 512], F32, tag="std16", name="std8k")

    def proj(w_sb, x_sb, dst, ssq_ps):
        # dst[:, d, :] = (x^T W)[d-block]^T staged bf16; ssq_ps[16, 2, 512]
        # accumulates per-head sums of squares via the sel16 selector.
        for d in range(8):
            accs = [ps_proj.tile([128, 512], F32, tag="acc", name=f"acc{c}")
                    for c in range(2)]
            for k in range(8):
                for c in range(2):
                    nc.tensor.matmul(
                        accs[c][:, :],
                        lhsT=w_sb[:, k, d * 128:(d + 1) * 128],
                        rhs=x_sb[:, k, c * 512:(c + 1) * 512],
                        start=(k == 0), stop=(k == 7),
                    )
            for c in range(2):
                staged = dst[:, d, c * 512:(c + 1) * 512]
                nc.scalar.copy(staged, accs[c][:, :])
                sq = sq_pool.tile([128, 512], F32R)
                nc.vector.tensor_mul(sq[:, :], staged, accs[c][:, :])
                nc.tensor.matmul(
                    ssq_ps[:, c, :],
                    lhsT=sel16_sb[:, d, :],
                    rhs=sq[:, :],
                    start=(d == 0), stop=(d == 7),
                    skip_group_check=True,
                )

    def rb_apply_one(d, dst, rb_ps, tag, rinv_sb, g_sb, b_sb, fast):
        # dst[:, d, :] *= selB-broadcast(rinv rows) (*g +b in the generic path)
        rb = rb_ps.tile([128, 2, 512], F32, tag=tag, name=f"rb{d}")
        for c in range(2):
            nc.tensor.matmul(
                rb[:, c, :],
                lhsT=selB_sb[:, d, :],
                rhs=rinv_sb[:, c, :],
                start=True, stop=True,
            )
        for c in range(2):
            dslice = dst[:, d, c * 512:(c + 1) * 512]
            if fast:
                nc.vector.tensor_mul(dslice, dslice, rb[:, c, :])
            else:
                nc.vector.scalar_tensor_tensor(
                    out=dslice, in0=dslice, scalar=g_sb[:, d:d + 1],
                    in1=rb[:, c, :],
                    op0=mybir.AluOpType.mult, op1=mybir.AluOpType.mult,
                )
            if b_sb is not None:
                nc.vector.tensor_scalar_add(dslice, dslice,
                                            b_sb[:, d:d + 1])

    # ---- P1: K projection ----
    ssq_k = ps_ssq.tile([16, 2, 512], F32, tag="ssq", name="ssq_k")
    proj(wk, enct, ktln, ssq_k)
    if fastk:
        # std8k = sqrt(ssq + 64*eps) = 8*std; exp scale uses 1/std8k = rinv/8
        nc.scalar.activation(std8k[:, :, :], ssq_k[:, :, :],
                             mybir.ActivationFunctionType.Sqrt,
                             bias=eps64_sb[:, :], scale=1.0)
    else:
        nc.scalar.activation(std8k[:, :, :], ssq_k[:, :, :],
                             mybir.ActivationFunctionType.Sqrt,
                             bias=eps_sb[:, :], scale=1.0 / HD)
        _recip_approx_r(nc, rinvk_t[:, :, :], std8k[:, :, :],
                        scr16[:, :, :])

    # ---- P1: Q projection (K's post-chain overlaps these matmuls) ----
    ssq_q = ps_ssq.tile([16, 2, 512], F32, tag="ssq", name="ssq_q")
    proj(wq, hst, qtln, ssq_q)
    nc.scalar.activation(stdq[:, :, :], ssq_q[:, :, :],
                         mybir.ActivationFunctionType.Sqrt,
                         bias=eps_sb[:, :], scale=1.0 / HD)
    _recip_approx_r(nc, rinvq[:, :, :], stdq[:, :, :], scr16[:, :, :])

    if fastk:
        # transpose std8k -> [kv, (vblock, head)] then one cheap reciprocal
        tps = ps_ssq.tile([128, 8, 16], F32, tag="ssq", name="tps")
        for vb in range(8):
            c, o = divmod(vb, 4)
            nc.tensor.transpose(tps[:, vb, :],
                                std8k[:, c, o * 128:(o + 1) * 128],
                                eye16_sb[:, :])
        nc.vector.reciprocal_approx_accurate(
            rinvk8[:, :, :], tps[:, :, :], scr128[:, :, :])
    else:
        for d in range(8):
            rb_apply_one(d, ktln, ps_ssq, "ssq", rinvk_t, gk_sb, bk_sb,
                         fast=False)

    def vproj(tt_lo, tt_hi, pool, tag):
        for tt in range(tt_lo, tt_hi):
            for c in range(2):
                vacc = pool.tile([128, 512], F32, tag=tag, name="vacc")
                for k in range(8):
                    nc.tensor.matmul(
                        vacc[:, :],
                        lhsT=enct[:, k, tt * 128:(tt + 1) * 128],
                        rhs=wv[:, k, c * 512:(c + 1) * 512],
                        start=(k == 0), stop=(k == 7),
                    )
                dst = vaug[:, tt, 8 * c:8 * (c + 1), 0:HD]
                nc.vector.tensor_copy(
                    dst, vacc[:, :].rearrange("p (h e) -> p h e", e=HD))

    # Q-side LN apply interleaved with the V-projection head start: the rb
    # ring (2 slots) would otherwise serialize PE against the vector
    # multiplies; V matmuls keep the PE streaming while they drain. Scores
    # for head-pair p only need feature block d==p, so the apply pipelines
    # ahead of attention.
    for d in range(8):
        rb_apply_one(d, qtln, ps_ssq, "ssq", rinvq, gq_sb, bq_sb, fast=fastq)
        if d % 3 == 1:
            vproj(d // 3, d // 3 + 1, ps_proj, "acc")
    p1.close()

    # ---- P2 pools (PSUM freed by p1.close(): sc 2 + av 4 + sums 2 banks) --
    at_pool = ctx.enter_context(tc.tile_pool(name="at_pool", bufs=3))
    p2 = ExitStack()
    ps_sc = p2.enter_context(tc.tile_pool(name="ps_sc", bufs=2, space="PSUM"))
    ps_av_cm = tc.tile_pool(name="ps_av", bufs=2, space="PSUM")
    ps_av = ps_av_cm.__enter__()

    # ---- P2: attention, software-pipelined (AV lags exp by one tile) ----
    pend_av = None

    def flush_av():
        nonlocal pend_av
        if pend_av is None:
            return
        avs, j, v, at = pend_av
        h = 2 * avs["p"] + j
        for qh in range(2):
            nc.tensor.matmul(
                avs[j][:, qh, :],
                lhsT=vaug[:, v, h, :],
                rhs=at[:, qh, :],
                start=(v == 0), stop=(v == 7),
                skip_group_check=True,
            )
        pend_av = None

    for p in range(8):
        avs = {j: ps_av.tile([HD + 1, 2, 512], F32, tag="av", name=f"av{j}")
               for j in range(2)}
        avs["p"] = p
        for v in range(8):
            if p == 0 and v >= 3:
                vproj(v, v + 1, ps_sc, "sc")  # finish V just-in-time
            for j in range(2):
                h = 2 * p + j
                scs = ps_sc.tile([128, 2, 512], F32, tag="sc")
                for qh in range(2):
                    nc.tensor.matmul(
                        scs[:, qh, :],
                        lhsT=ktln[j * 64:(j + 1) * 64, p,
                                  v * 128:(v + 1) * 128],
                        rhs=qtln[j * 64:(j + 1) * 64, p,
                                 qh * 512:(qh + 1) * 512],
                        start=True, stop=True,
                    )
                flush_av()
                at = at_pool.tile([128, 2, 512], BF)
                scale = rinvk8[:, v, h:h + 1] if fastk else 0.125
                nc.scalar.activation(
                    at[:, :, :], scs[:, :, :],
                    mybir.ActivationFunctionType.Exp, scale=scale,
                )
                pend_av = (avs, j, v, at)
        flush_av()
        sums_p = ps_av.tile([16, 2, 512], F32, tag="av", name="sums_p")
        for j in range(2):
            h = 2 * p + j
            nc.vector.tensor_copy(aout[j * 64:(j + 1) * 64, p, :],
                                  avs[j][0:HD, :, :])
            srow = srow_pool.tile([1, 2, 512], F32R)
            with nc.allow_low_precision(reason="f32r softmax sum staging"):
                nc.vector.tensor_copy(srow[:, :, :], avs[j][HD:HD + 1, :, :])
            for c in range(2):
                nc.tensor.matmul(
                    sums_p[:, c, :],
                    lhsT=selE_sb[:, h, :],
                    rhs=srow[:, c, :],
                    start=(j == 0), stop=(j == 1),
                    skip_group_check=True,
                )
        nc.vector.tensor_add(sums_sb[:, :, :], sums_sb[:, :, :],
                             sums_p[:, :, :])

    # ---- P2.5/P3: normalize by softmax sums and project, per token-half;
    # the av pool closes first so the out-projection PSUM can open while the
    # second half still normalizes.
    ps_av_cm.__exit__(None, None, None)
    out_pool = ctx.enter_context(tc.tile_pool(name="out_pool", bufs=4))
    ps_out_cm = tc.tile_pool(name="ps_out", bufs=4, space="PSUM")
    ps_out = ps_out_cm.__enter__()

    def normalize_half(c):
        _recip_approx_r(nc, invs[:, c, :], sums_sb[:, c, :], scr16[:, c, :])
        for d in range(8):
            rbi = ps_sc.tile([128, 512], F32, tag="sc", name="rbi")
            nc.tensor.matmul(
                rbi[:, :],
                lhsT=selB_sb[:, d, :],
                rhs=invs[:, c, :],
                start=True, stop=True,
            )
            sl = aout[:, d, c * 512:(c + 1) * 512]
            nc.vector.tensor_mul(sl, sl, rbi[:, :])

    def oproj_half(c):
        for tt in range(4 * c, 4 * (c + 1)):
            accs = {cc: ps_out.tile([128, 512], F32, tag="oacc",
                                    name=f"oacc{cc}") for cc in range(2)}
            for k in range(8):
                for cc in range(2):
                    nc.tensor.matmul(
                        accs[cc][:, :],
                        lhsT=aout[:, k, tt * 128:(tt + 1) * 128],
                        rhs=wo[:, k, cc * 512:(cc + 1) * 512],
                        start=(k == 0), stop=(k == 7),
                    )
            for cc in range(2):
                ot = out_pool.tile([128, 512], F32)
                nc.scalar.copy(ot[:, :], accs[cc][:, :])
                nc.sync.dma_start(
                    t["out"][tt * 128:(tt + 1) * 128,
                             cc * 512:(cc + 1) * 512],
                    ot[:, :],
                )

    normalize_half(0)
    oproj_half(0)
    normalize_half(1)
    oproj_half(1)
    ps_out_cm.__exit__(None, None, None)
    p2.close()


def _build(fastq, fastk, has_bq, has_bk):
    key = (fastq, fastk, has_bq, has_bk)
    if key in _cache:
        return _cache[key]
    nc = bacc.Bacc("TRN2", target_bir_lowering=False, debug=False,
                   num_devices=NCORES)
    t = {}

    def inp(name, shape, dt):
        t[name] = nc.dram_tensor(name, list(shape), dt, kind="ExternalInput").ap()

    inp("hsT", (D, M), BF)
    inp("encT", (D, SKV), BF)
    inp("wq", (D, D), BF)
    inp("wk", (D, D), BF)
    inp("wv", (D, D), BF)
    inp("wo", (D, D), BF)
    if (not fastq) or (not fastk) or has_bq or has_bk:
        inp("gq", (D,), F32)
        inp("gk", (D,), F32)
    if has_bq:
        inp("bq", (D,), F32)
    if has_bk:
        inp("bk", (D,), F32)
    inp("sel16", (8, 128, H), F32R)
    inp("selB", (8, H, 128), F32R)
    inp("selE", (1, H, H), F32R)
    inp("eye16", (16, 16), F32)
    t["out"] = nc.dram_tensor("out", [M, D], F32, kind="ExternalOutput").ap()

    with tile.TileContext(nc) as tc:
        with ExitStack() as ctx:
            _emit(ctx, tc, t, fastq, fastk, has_bq, has_bk)
    nc.finalize()
    _cache[key] = nc
    return nc


def _center_fold(W):
    # Fold per-head output-column mean removal into the weight matrix (exact).
    Wr = np.asarray(W, np.float32).reshape(D, H, HD)
    return (Wr - Wr.mean(axis=2, keepdims=True)).reshape(D, D)


def kernel(hidden_states, encoder_hidden_states, Wq, Wk, Wv, Wo,
           gq, bq, gk, bk, _trace=False):
    hs = np.asarray(hidden_states, np.float32)
    enc = np.asarray(encoder_hidden_states, np.float32)
    gq = np.asarray(gq, np.float32)
    gk = np.asarray(gk, np.float32)
    bq = np.asarray(bq, np.float32)
    bk = np.asarray(bk, np.float32)
    has_bq = bool(np.any(bq != 0))
    has_bk = bool(np.any(bk != 0))
    fastq = bool(np.all(gq == 1.0)) and not has_bq
    fastk = bool(np.all(gk == 1.0)) and not has_bk
    nc = _build(fastq, fastk, has_bq, has_bk)

    bf = ml_dtypes.bfloat16
    sel16, selB, selE = _selector_constants()
    common = {
        "wq": _center_fold(Wq).astype(bf),
        "wk": _center_fold(Wk).astype(bf),
        "wv": np.asarray(Wv, np.float32).astype(bf),
        "wo": np.asarray(Wo, np.float32).astype(bf),
        "sel16": sel16, "selB": selB, "selE": selE,
        "eye16": np.eye(16, dtype=np.float32),
    }
    if (not fastq) or (not fastk) or has_bq or has_bk:
        common["gq"] = np.tile(gq, H)
        common["gk"] = np.tile(gk, H)
    if has_bq:
        common["bq"] = np.tile(bq, H)
    if has_bk:
        common["bk"] = np.tile(bk, H)

    in_maps = []
    for core in range(NCORES):
        b, qb = divmod(core, 4)
        hsT = np.ascontiguousarray(
            hs[b, qb * M:(qb + 1) * M, :].T).astype(bf)
        encT = np.ascontiguousarray(enc[b].T).astype(bf)
        in_maps.append({**common, "hsT": hsT, "encT": encT})

    res = run_bass_kernel_spmd(nc, in_maps, list(range(NCORES)), trace=_trace)

    out = np.empty((B, SQ, D), np.float32)
    for core in range(NCORES):
        b, qb = divmod(core, 4)
        out[b, qb * M:(qb + 1) * M, :] = res.results[core]["out"]
    kernel.last_exec_time_ns = res.exec_time_ns
    kernel.last_results = res
    return out


# revision 32
# speedup vs baseline: 1.4861x; 1.1280x over previous
"""CrossAttention Trainium2 kernel (8 NeuronCores, SPMD).

Sharding: 8 cores = batch(2) x query-block(4 x 1024). Each core computes a
[1024, 1024] slice of the output; no cross-core communication.

Reference math (per core, M=1024 query tokens, Skv=1024, D=1024, H=16, hd=64):
  q = hs @ Wq ; k = enc @ Wk ; v = enc @ Wv
  per-head LN(q), LN(k) over hd; scores = LN(q) @ LN(k)^T / sqrt(hd)
  out = softmax(scores) @ v ; return out @ Wo

Host folds the LN mean-centering into Wq/Wk (exact), pre-transposes
activations to feature-major, and casts matmul operands to bf16.
"""

import numpy as np
import ml_dtypes
from contextlib import ExitStack

import concourse.bass as bass
import concourse.tile as tile
from concourse import bacc, mybir
from concourse.bass_utils import run_bass_kernel_spmd
from concourse.dve_ops import (
    RECIP_APPROX_FAST_CONSTS,
    RECIPROCAL_APPROX_FAST,
    RECIPROCAL_APPROX_NR,
)


def _recip_approx_r(nc, out, in_, scratch):
    """~2 ULP approximate 1/in_ written as FP32R, ~2.8x faster than
    nc.vector.reciprocal; shortens the LN/softmax dependency chains that
    otherwise idle the PE long enough to trip the HAM clock throttle."""
    c = RECIP_APPROX_FAST_CONSTS
    nc.vector._custom_dve(RECIPROCAL_APPROX_FAST, out=scratch, in0=in_,
                          s0=c["s0"], s1=c["s1"], imm2=c["imm2"])
    nc.vector._custom_dve(RECIPROCAL_APPROX_NR, out=out, in0=in_,
                          in1=scratch, s0=2.0)

BF = mybir.dt.bfloat16
F32 = mybir.dt.float32
F32R = mybir.dt.float32r

D = 1024      # model dim
H = 16        # heads
HD = 64       # head dim
M = 1024      # query tokens per core
SKV = 1024    # kv tokens (one batch)
B = 2
SQ = 4096
NCORES = 8
LN_EPS = 1e-5

_cache = {}


def _selector_constants():
    # sel16[d][p, j]: 1 if head j == 2d + p//64  (sum-over-head-partitions lhsT)
    sel16 = np.zeros((8, 128, H), np.float32)
    for d in range(8):
        for p in range(128):
            sel16[d, p, 2 * d + p // 64] = 1.0
    # selB[d][j, p]: 1 if head j == 2d + p//64  (broadcast-to-head-partitions lhsT)
    selB = np.transpose(sel16, (0, 2, 1)).copy()
    # selE[0][h][j]: identity — scatter a [1, CH] row into head h of [16, CH]
    selE = np.eye(H, dtype=np.float32)[None, :, :]
    return sel16, selB, selE


def _emit(ctx: ExitStack, tc, t, has_bias_q, has_bias_k):
    nc = tc.nc

    persist = ctx.enter_context(tc.tile_pool(name="persist", bufs=1))

    # ---- persistent SBUF tensors (stacked [128, 8, 1024] layout) ----
    hst = persist.tile([128, 8, M], BF, tag="hst")       # hs^T  (feature-major)
    enct = persist.tile([128, 8, SKV], BF, tag="enct")   # enc^T (feature-major)
    wq = persist.tile([128, 8, D], BF, tag="wq")
    wk = persist.tile([128, 8, D], BF, tag="wk")
    wv = persist.tile([128, 8, D], BF, tag="wv")
    wo = persist.tile([128, 8, D], BF, tag="wo")
    qtln = persist.tile([128, 8, M], BF, tag="qtln")     # LN(q)^T feature-major
    ktln = persist.tile([128, 8, SKV], BF, tag="ktln")   # LN(k)^T feature-major
    vaug = persist.tile([128, 8, H, HD + 1], BF, tag="vaug")  # [kv, h, V|1]
    aout = persist.tile([128, 8, M], BF, tag="aout")     # attn out^T feature-major
    gq_sb = persist.tile([128, 8], F32, tag="gq_sb")
    gk_sb = persist.tile([128, 8], F32, tag="gk_sb")
    sel16_sb = persist.tile([128, 8, H], F32R, tag="sel16_sb")
    selB_sb = persist.tile([16, 8, 128], F32R, tag="selB_sb")
    rinv_q = persist.tile([16, M], F32R, tag="rinv_q")    # 1/std per (head, tok)
    rinv_k = persist.tile([16, SKV], F32R, tag="rinv_k")
    inv_s = persist.tile([16, M], F32R, tag="inv_s")      # 1/softmax-sum
    selE_sb = persist.tile([1, H, H], F32R, tag="selE_sb")
    eps_sb = persist.tile([16, 1], F32, tag="eps_sb")
    rscr_sb = persist.tile([16, 512], F32, tag="rscr_sb")
    nc.vector.memset(eps_sb[:, :], LN_EPS)
    nc.vector.memset(vaug[:, :, :, HD:HD + 1], 1.0)
    bq_sb = persist.tile([128, 8], F32, tag="bq_sb") if has_bias_q else None
    bk_sb = persist.tile([128, 8], F32, tag="bk_sb") if has_bias_k else None

    # ---- loads ----
    for k in range(8):
        nc.sync.dma_start(enct[:, k, :], t["encT"][k * 128:(k + 1) * 128, :])
        nc.sync.dma_start(wk[:, k, :], t["wk"][k * 128:(k + 1) * 128, :])
    for k in range(8):
        nc.sync.dma_start(wv[:, k, :], t["wv"][k * 128:(k + 1) * 128, :])
        nc.sync.dma_start(hst[:, k, :], t["hsT"][k * 128:(k + 1) * 128, :])
    for k in range(8):
        nc.sync.dma_start(wq[:, k, :], t["wq"][k * 128:(k + 1) * 128, :])
        nc.sync.dma_start(wo[:, k, :], t["wo"][k * 128:(k + 1) * 128, :])
    nc.sync.dma_start(gq_sb[:, :], t["gq"].rearrange("(d p) -> p d", p=128))
    nc.sync.dma_start(gk_sb[:, :], t["gk"].rearrange("(d p) -> p d", p=128))
    if has_bias_q:
        nc.sync.dma_start(bq_sb[:, :], t["bq"].rearrange("(d p) -> p d", p=128))
    if has_bias_k:
        nc.sync.dma_start(bk_sb[:, :], t["bk"].rearrange("(d p) -> p d", p=128))
    nc.sync.dma_start(sel16_sb[:, :, :], t["sel16"].rearrange("d p j -> p d j"))
    nc.sync.dma_start(selB_sb[:, :, :], t["selB"].rearrange("d j p -> j d p"))
    nc.sync.dma_start(selE_sb[:, :, :], t["selE"])

    # ---- P1: projection + per-head LN for Q and K ----
    sq_pool = ctx.enter_context(tc.tile_pool(name="sq_pool", bufs=2))
    rstd_pool = ctx.enter_context(tc.tile_pool(name="rstd_pool", bufs=2))

    def proj_ln(ps_proj, ps_ssq, ps_rb, w_sb, x_sb, ln_sb, g_sb, b_sb,
                rinv_sb, ntok, split_c=False, stage_act=False):
        nchunk = ntok // 512
        c_groups = [[c] for c in range(nchunk)] if split_c else [list(range(nchunk))]
        for cg in c_groups:
            ssqs = {}
            for c in cg:
                ssqs[c] = ps_ssq.tile([16, 512], F32, tag="ssq", name=f"ssq{c}")
            for d in range(8):
                accs = {c: ps_proj.tile([128, 512], F32, tag="acc",
                                        name=f"acc{c}") for c in cg}
                for k in range(8):
                    for c in cg:
                        nc.tensor.matmul(
                            accs[c][:, :],
                            lhsT=w_sb[:, k, d * 128:(d + 1) * 128],
                            rhs=x_sb[:, k, c * 512:(c + 1) * 512],
                            start=(k == 0), stop=(k == 7),
                        )
                for c in cg:
                    acc = accs[c]
                    # stage raw projection (bf16); LN apply rescales in place
                    if stage_act:
                        nc.scalar.copy(ln_sb[:, d, c * 512:(c + 1) * 512],
                                       acc[:, :])
                    else:
                        nc.vector.tensor_copy(
                            ln_sb[:, d, c * 512:(c + 1) * 512], acc[:, :])
                    # squares from staged bf16; per-head sum via selector
                    sq = sq_pool.tile([128, 512], F32R)
                    nc.vector.tensor_mul(sq[:, :],
                                         ln_sb[:, d, c * 512:(c + 1) * 512],
                                         ln_sb[:, d, c * 512:(c + 1) * 512])
                    nc.tensor.matmul(
                        ssqs[c][:, :],
                        lhsT=sel16_sb[:, d, :],
                        rhs=sq[:, :],
                        start=(d == 0), stop=(d == 7),
                        skip_group_check=True,
                    )
            for c in cg:
                ssq = ssqs[c]
                # rstd = 1/sqrt(ssq/64 + eps)
                rstd = rstd_pool.tile([16, 512], F32)
                nc.scalar.activation(
                    rstd[:, :], ssq[:, :], mybir.ActivationFunctionType.Sqrt,
                    bias=eps_sb[:, :], scale=1.0 / HD,
                )
                _recip_approx_r(nc, rinv_sb[:, c * 512:(c + 1) * 512],
                                rstd[:, :], rscr_sb[:, :])
                # apply: ln = raw * g * rinv (+ b)
                for d in range(8):
                    rb = ps_rb.tile([128, 512], F32)
                    nc.tensor.matmul(
                        rb[:, :],
                        lhsT=selB_sb[:, d, :],
                        rhs=rinv_sb[:, c * 512:(c + 1) * 512],
                        start=True, stop=True,
                    )
                    dst = ln_sb[:, d, c * 512:(c + 1) * 512]
                    nc.vector.scalar_tensor_tensor(
                        out=dst,
                        in0=dst,
                        scalar=g_sb[:, d:d + 1],
                        in1=rb[:, :],
                        op0=mybir.AluOpType.mult,
                        op1=mybir.AluOpType.mult,
                    )
                    if b_sb is not None:
                        nc.vector.tensor_scalar_add(dst, dst,
                                                    b_sb[:, d:d + 1])

    with tc.tile_pool(name="ps_proj", bufs=4, space="PSUM") as ps_proj, \
         tc.tile_pool(name="ps_ssq", bufs=2, space="PSUM") as ps_ssq, \
         tc.tile_pool(name="ps_rb", bufs=2, space="PSUM") as ps_rb:
        proj_ln(ps_proj, ps_ssq, ps_rb, wk, enct, ktln, gk_sb, bk_sb, rinv_k,
                SKV, stage_act=True)

        # ---- P1b: V projection into augmented layout [kv, h, V|1] ----
        for tt in range(8):
            accs = [ps_proj.tile([128, 512], F32, tag="acc", name=f"acc{i}")
                    for i in range(2)]
            for k in range(8):
                for c in range(2):
                    nc.tensor.matmul(
                        accs[c][:, :],
                        lhsT=enct[:, k, tt * 128:(tt + 1) * 128],
                        rhs=wv[:, k, c * 512:(c + 1) * 512],
                        start=(k == 0), stop=(k == 7),
                    )
            for c in range(2):
                dst = vaug[:, tt, 8 * c:8 * (c + 1), 0:HD]
                nc.scalar.copy(
                    dst, accs[c][:, :].rearrange("p (h e) -> p h e", e=HD))

        # Q last, chunk-split, so attention on chunk 0 overlaps chunk 1
        proj_ln(ps_proj, ps_ssq, ps_rb, wq, hst, qtln, gq_sb, bq_sb, rinv_q,
                M, split_c=True)

    # ---- P2: attention ----
    at_pool = ctx.enter_context(tc.tile_pool(name="at_pool", bufs=4))
    CH = 512  # query-token chunk
    p2 = ExitStack()
    ps_sc = p2.enter_context(tc.tile_pool(name="ps_sc", bufs=2, space="PSUM"))
    ps_av = p2.enter_context(tc.tile_pool(name="ps_av", bufs=2, space="PSUM"))
    ps_sum = p2.enter_context(tc.tile_pool(name="ps_sum", bufs=1, space="PSUM"))
    srow_pool = ctx.enter_context(tc.tile_pool(name="srow_pool", bufs=2))

    for c in range(M // CH):
        sums = ps_sum.tile([16, CH], F32)
        for p in range(8):
            avs = {j: ps_av.tile([HD + 1, CH], F32, tag="av", name=f"av{j}")
                   for j in range(2)}
            for quarter in range(4):
                scs = {j: ps_sc.tile([128, 2, CH], F32, tag="sc",
                                     name=f"sc{j}") for j in range(2)}
                for vv in range(2):
                    v = 2 * quarter + vv
                    for j in range(2):
                        nc.tensor.matmul(
                            scs[j][:, vv, :],
                            lhsT=ktln[j * 64:(j + 1) * 64, p,
                                      v * 128:(v + 1) * 128],
                            rhs=qtln[j * 64:(j + 1) * 64, p,
                                     c * CH:(c + 1) * CH],
                            start=True, stop=True,
                        )
                for j in range(2):
                    at = at_pool.tile([128, 2, CH], BF)
                    nc.scalar.activation(
                        at[:, :, :], scs[j][:, :, :],
                        mybir.ActivationFunctionType.Exp, scale=0.125,
                    )
                    for vv in range(2):
                        v = 2 * quarter + vv
                        nc.tensor.matmul(
                            avs[j][:, :],
                            lhsT=vaug[:, v, 2 * p + j, :],
                            rhs=at[:, vv, :],
                            start=(v == 0), stop=(v == 7),
                            skip_group_check=True,
                        )
            for j in range(2):
                h = 2 * p + j
                av = avs[j]
                nc.vector.tensor_copy(
                    aout[j * 64:(j + 1) * 64, p, c * CH:(c + 1) * CH],
                    av[0:HD, :])
                srow = srow_pool.tile([1, CH], F32R)
                with nc.allow_low_precision(reason="f32r staging"):
                    nc.vector.tensor_copy(srow[:, :], av[HD:HD + 1, :])
                nc.tensor.matmul(
                    sums[:, :],
                    lhsT=selE_sb[:, h, :],
                    rhs=srow[:, :],
                    start=(h == 0), stop=(h == 15),
                    skip_group_check=True,
                )
        _recip_approx_r(nc, inv_s[:, c * CH:(c + 1) * CH], sums[:, :],
                        rscr_sb[:, :])
    p2.close()

    # ---- P2.5: normalize attention output ----
    out_pool = ctx.enter_context(tc.tile_pool(name="out_pool", bufs=4))
    with tc.tile_pool(name="ps_rb2", bufs=3, space="PSUM") as ps_rb2, \
         tc.tile_pool(name="ps_out", bufs=4, space="PSUM") as ps_out:
        for c in range(M // CH):
            for p in range(8):
                rb = ps_rb2.tile([128, CH], F32)
                nc.tensor.matmul(
                    rb[:, :],
                    lhsT=selB_sb[:, p, :],
                    rhs=inv_s[:, c * CH:(c + 1) * CH],
                    start=True, stop=True,
                )
                sl = aout[:, p, c * CH:(c + 1) * CH]
                nc.vector.tensor_mul(sl, sl, rb[:, :])

            # ---- P3: output projection for this chunk's token tiles ----
            for tt in range(4 * c, 4 * (c + 1)):
                accs = {cc: ps_out.tile([128, 512], F32, tag="oacc",
                                        name=f"oacc{cc}") for cc in range(2)}
                for k in range(8):
                    for cc in range(2):
                        nc.tensor.matmul(
                            accs[cc][:, :],
                            lhsT=aout[:, k, tt * 128:(tt + 1) * 128],
                            rhs=wo[:, k, cc * 512:(cc + 1) * 512],
                            start=(k == 0), stop=(k == 7),
                        )
                for cc in range(2):
                    ot = out_pool.tile([128, 512], F32)
                    nc.scalar.copy(ot[:, :], accs[cc][:, :])
                    nc.sync.dma_start(
                        t["out"][tt * 128:(tt + 1) * 128,
                                 cc * 512:(cc + 1) * 512],
                        ot[:, :],
                    )


def _build(has_bias_q, has_bias_k):
    key = (has_bias_q, has_bias_k)
    if key in _cache:
        return _cache[key]
    nc = bacc.Bacc("TRN2", target_bir_lowering=False, debug=False,
                   num_devices=NCORES)
    bf_np = np.dtype(ml_dtypes.bfloat16)
    t = {}

    def inp(name, shape, dt):
        t[name] = nc.dram_tensor(name, list(shape), dt, kind="ExternalInput").ap()

    inp("hsT", (D, M), BF)
    inp("encT", (D, SKV), BF)
    inp("wq", (D, D), BF)
    inp("wk", (D, D), BF)
    inp("wv", (D, D), BF)
    inp("wo", (D, D), BF)
    inp("gq", (D,), F32)
    inp("gk", (D,), F32)
    if has_bias_q:
        inp("bq", (D,), F32)
    if has_bias_k:
        inp("bk", (D,), F32)
    inp("sel16", (8, 128, H), F32R)
    inp("selB", (8, H, 128), F32R)
    inp("selE", (1, H, H), F32R)
    t["out"] = nc.dram_tensor("out", [M, D], F32, kind="ExternalOutput").ap()

    with tile.TileContext(nc) as tc:
        with ExitStack() as ctx:
            _emit(ctx, tc, t, has_bias_q, has_bias_k)
    nc.finalize()
    _cache[key] = nc
    return nc


def _center_fold(W):
    # Fold per-head output-column mean removal into the weight matrix (exact).
    Wr = np.asarray(W, np.float32).reshape(D, H, HD)
    return (Wr - Wr.mean(axis=2, keepdims=True)).reshape(D, D)


def kernel(hidden_states, encoder_hidden_states, Wq, Wk, Wv, Wo,
           gq, bq, gk, bk, _trace=False):
    hs = np.asarray(hidden_states, np.float32)
    enc = np.asarray(encoder_hidden_states, np.float32)
    bq = np.asarray(bq, np.float32)
    bk = np.asarray(bk, np.float32)
    has_bias_q = bool(np.any(bq != 0))
    has_bias_k = bool(np.any(bk != 0))
    nc = _build(has_bias_q, has_bias_k)

    bf = ml_dtypes.bfloat16
    wq_bf = _center_fold(Wq).astype(bf)
    wk_bf = _center_fold(Wk).astype(bf)
    wv_bf = np.asarray(Wv, np.float32).astype(bf)
    wo_bf = np.asarray(Wo, np.float32).astype(bf)
    gq_rep = np.tile(np.asarray(gq, np.float32), H)
    gk_rep = np.tile(np.asarray(gk, np.float32), H)
    sel16, selB, selE = _selector_constants()

    common = {
        "wq": wq_bf, "wk": wk_bf, "wv": wv_bf, "wo": wo_bf,
        "gq": gq_rep, "gk": gk_rep,
        "sel16": sel16, "selB": selB, "selE": selE,
    }
    if has_bias_q:
        common["bq"] = np.tile(bq, H)
    if has_bias_k:
        common["bk"] = np.tile(bk, H)

    in_maps = []
    for core in range(NCORES):
        b, qb = divmod(core, 4)
        hsT = np.ascontiguousarray(
            hs[b, qb * M:(qb + 1) * M, :].T).astype(bf)
        encT = np.ascontiguousarray(enc[b].T).astype(bf)
        in_maps.append({**common, "hsT": hsT, "encT": encT})

    res = run_bass_kernel_spmd(nc, in_maps, list(range(NCORES)), trace=_trace)

    out = np.empty((B, SQ, D), np.float32)
    for core in range(NCORES):
        b, qb = divmod(core, 4)
        out[b, qb * M:(qb + 1) * M, :] = res.results[core]["out"]
    kernel.last_exec_time_ns = res.exec_time_ns
    kernel.last_results = res
    return out

